# revision 41
# baseline (speedup 1.0000x reference)
"""Trainium2 Bass kernel for nn_Attention_53687091200195 (v3).

Reference computation (per batch b):
    Q = relu(x @ Wq + bq); K = relu(x @ Wk + bk); V = relu(x @ Wv + bv)
    S = Q @ K^T / sqrt(64); P = softmax(S, axis=-1); out = P @ V

Shapes: x [16, 2048, 64] f32, W* [64, 128] f32, b* [128] f32 -> out [16, 2048, 128].
Sharding: data-parallel over batch. 8 cores x 2 batches each; weights replicated.

v3 design (v0 baseline ~152us):
  - Token-permuted layout (internal token ntilde = j*128 + p maps to real
    token p*16 + j) keeps every DMA per-partition contiguous; attention is
    permutation-equivariant, the host undoes the permutation (pure layout).
  - Projections in f32r with bias folding (65-row contraction, ones row).
  - Scores S^T = K_m @ Q^T in f32r (16 m-tiles x 2 halves per 1024-q chunk).
  - E = exp(S/8) stored fp8e4m3 BYTE-INTERLEAVED in key-tile pairs:
    e8i[p, mhat, q, o] with o = m%2 at byte stride 2, so the DoubleRow
    matmuls' moving operand reads one 16-bit pair per lane per cycle ->
    2 fp8 MACs/cell/cycle.  PV (out^T += V_pair^T @ E_pair) and the softmax
    denominator (ones_pair^T @ E_pair) each take 16 512-col passes per chunk
    at ~2x the f32r MAC rate (vs 32 passes each in v0).
  - exp split across engines: ACT does exact exp->fp8 for 9/16 tiles per
    chunk, DVE synthesizes fp8 bits directly via a Schraudolph construction
    (int8(S*log2e + 55.63) IS the fp8e4m3 bit pattern of ~exp(S/8)) for 7.
  - No PE out-transposes: out^T [d, q] is normalized via a K=1 ones-matmul
    broadcast of den into PSUM, reciprocal_approx_fast, and tensor_tensor
    multiplies; stored transposed + permuted, host fixes layout.
  - den accumulates in a single PSUM bank ([33, 512] tile, query-half 1 at
    partition 32 via col-group tiling), freeing a bank for a dedicated
    broadcast pool so the epilogue never blocks the score pipeline.
  - PE warmup on zeroed f32r tiles + dummy exp during the initial x DMA so
    HAM reaches 2.4 GHz and the ACT exp table is resident before real work.
  - fp8 quantization of E/V adds ~0.6% output error (tolerance 2e-2).
"""

import numpy as np

import concourse.bass as bass
import concourse.mybir as mybir
import concourse.tile as tile
from concourse import bacc
from concourse.bass_utils import run_bass_kernel_spmd

N_CORES = 8
B_PER_CORE = 2
N_TOK = 2048
C_IN = 64
D = 128
P = 128
N_TILES = N_TOK // P          # 16
N_CHUNK = 1024
N_CHUNKS = N_TOK // N_CHUNK   # 2
MHAT = N_TILES // 2           # 8 pair-tiles per chunk
SCALE = 0.125                 # 1/sqrt(64)
LOG2E = 1.4426950408889634
B8 = 55.63                    # Schraudolph offset for fp8e4m3 bits

F32 = mybir.dt.float32
F32R = mybir.dt.float32r
FP8 = mybir.dt.float8e4
I8 = mybir.dt.int8
DR = mybir.MatmulPerfMode.DoubleRow

# exp engine per m-tile within a chunk: True -> ACT, False -> DVE (10/6
# split: strided DVE fp8 writes measure ~1.56us vs ACT's ~1.15us; the two
# both-ACT pairs sit apart so the pst rotation can absorb their serialization)
EXP_ON_ACT = [True, False] * (N_TILES // 2)
EXP_ON_ACT[15] = True


def pair_ap(e_slice):
    """Reorder a [128, N, 2] fp8 AP into DoubleRow's [K, 2, N] operand shape
    (dim1 = pair member at byte step 1, dim2 = columns at byte step 2)."""
    return bass.AP(
        tensor=e_slice.tensor,
        offset=e_slice.offset,
        ap=[e_slice.ap[0], e_slice.ap[2], e_slice.ap[1]],
    )


def build_program():
    nc = bacc.Bacc("TRN2", target_bir_lowering=False, debug=False,
                   num_devices=N_CORES)

    x = nc.dram_tensor("x", [B_PER_CORE, N_TOK, C_IN], F32, kind="ExternalInput").ap()
    wq = nc.dram_tensor("Wq", [C_IN, D], F32, kind="ExternalInput").ap()
    bq = nc.dram_tensor("bq", [D], F32, kind="ExternalInput").ap()
    wk = nc.dram_tensor("Wk", [C_IN, D], F32, kind="ExternalInput").ap()
    bk = nc.dram_tensor("bk", [D], F32, kind="ExternalInput").ap()
    wv = nc.dram_tensor("Wv", [C_IN, D], F32, kind="ExternalInput").ap()
    bv = nc.dram_tensor("bv", [D], F32, kind="ExternalInput").ap()
    # transposed + token-permuted output; host untangles the layout
    out = nc.dram_tensor("out", [B_PER_CORE, D, N_TOK], F32, kind="ExternalOutput").ap()

    with tile.TileContext(nc) as tc:
        kernel_body(tc, out, x, (wq, bq), (wk, bk), (wv, bv))

    nc.compile()
    return nc


def kernel_body(tc, out, x, qw, kw, vw):
    nc = tc.nc
    from contextlib import ExitStack
    ctx = ExitStack()
    with ctx:
        consts = ctx.enter_context(tc.tile_pool(name="consts", bufs=1))
        perb = ctx.enter_context(tc.tile_pool(name="perb", bufs=2))
        epool = ctx.enter_context(tc.tile_pool(name="epool", bufs=1))
        ep = ctx.enter_context(tc.tile_pool(name="ep", bufs=2))

        # --- warmup constants first: only DVE memsets in their deps ---
        warm_st = consts.tile([P, P], F32R, name="warm_st", tag="warm_st")
        nc.vector.memset(warm_st[:].bitcast(F32), 0.0)
        warm_mov = consts.tile([P, 512], F32R, name="warm_mov", tag="warm_mov")
        nc.vector.memset(warm_mov[:].bitcast(F32), 0.0)

        # --- constants ---
        identity = consts.tile([P, P], F32)
        nc.vector.memset(identity[:], 0.0)
        nc.gpsimd.affine_select(
            out=identity[:], in_=identity[:],
            compare_op=mybir.AluOpType.not_equal, fill=1.0,
            base=0, pattern=[[-1, P]], channel_multiplier=1)
        ones_bc_f = consts.tile([1, P], F32)
        nc.vector.memset(ones_bc_f[:], 1.0)
        ones_bc = consts.tile([1, P], F32R)
        nc.vector.tensor_copy(out=ones_bc[:], in_=ones_bc_f[:])
        ones8 = consts.tile([P, 2, 16], FP8)
        nc.vector.memset(ones8[:], 1.0)

        # x for both batches, token-permuted: x_nat2[p, j, b, c] = x[b, p*16+j, c]
        x_nat2 = consts.tile([P, N_TILES, B_PER_CORE, C_IN], F32, name="x_nat2",
                             tag="x_nat2")
        # 6 DMAs across 3 queues: x fully resident by ~6us so the transpose
        # stream never stalls mid-phase-A (which would re-throttle HAM)
        H = N_TILES // 3 if N_TILES % 3 == 0 else 6
        qengs = (nc.sync, nc.scalar, nc.gpsimd)
        bounds = [0, 6, 11, 16]
        for jh in range(3):
            j0, j1 = bounds[jh], bounds[jh + 1]
            for bb in range(B_PER_CORE):
                qengs[jh].dma_start(
                    out=x_nat2[:, j0:j1, bb, :],
                    in_=bass.AP(
                        tensor=x.tensor,
                        offset=bb * N_TOK * C_IN + j0 * C_IN,
                        ap=[[N_TILES * C_IN, P], [C_IN, j1 - j0], [1, C_IN]],
                    ),
                )

        # Bias folding: row 64 of xT is ones, row 64 of each weight is bias.
        w_sb = {}
        for name, (w, b) in (("q", qw), ("k", kw), ("v", vw)):
            wf = consts.tile([C_IN, D], F32, name=f"wf_{name}", tag=f"wf_{name}")
            nc.scalar.dma_start(out=wf[:], in_=w[:])
            bf = consts.tile([1, D], F32, name=f"bf_{name}", tag=f"bf_{name}")
            nc.scalar.dma_start(out=bf[:], in_=b[:])
            w2 = consts.tile([C_IN + 1, D], F32R, name=f"w_{name}", tag=f"w_{name}")
            nc.vector.tensor_copy(out=w2[0:C_IN, :], in_=wf[:])
            nc.vector.tensor_copy(out=w2[C_IN:C_IN + 1, :], in_=bf[:])
            w_sb[name] = w2

        xTs = [perb.tile([C_IN + 1, N_TOK], F32R, name=f"xT_{bb}",
                         tag=f"xT_{bb}", bufs=1)
               for bb in range(B_PER_CORE)]
        for bb in range(B_PER_CORE):
            nc.gpsimd.memset(xTs[bb][C_IN:C_IN + 1, :].bitcast(F32), 1.0)

        qTs, kTs = {}, {}
        v8s = [perb.tile([P, N_TILES, D], FP8, name=f"v8_{bb}",
                         tag=f"v8_{bb}", bufs=1)
               for bb in range(B_PER_CORE)]

        # ---------------- Phase A: prologue (own PSUM scope) ----------------
        with tc.tile_pool(name="ptr", bufs=3, space="PSUM") as ptr, \
             tc.tile_pool(name="ppj", bufs=2, space="PSUM") as ppj, \
             tc.tile_pool(name="pwarm", bufs=1, space="PSUM") as pwarm:

            # PE warmup: cold matmuls during the x DMA flip HAM to K=8/8
            # (2.4 GHz); fillers between the transposes keep the duty cycle
            # above HAM's re-throttle threshold through sparse phase A.  The
            # warm tile has no readers, so fillers inherit only PE-internal
            # WAW order and genuinely absorb idle gaps (x is fully resident
            # by ~14us on 3 DMA queues, so the transposes ahead of them
            # never stall on HBM).
            warm = pwarm.tile([P, 512], F32, tag="warm", name="warm")

            def warm_mm():
                nc.tensor.matmul(warm[:], warm_st[:], warm_mov[:],
                                 start=True, stop=True, skip_group_check=True)

            for i in range(12):
                warm_mm()
                if i == 0:
                    # dummy exp: ACT table load off the critical path
                    dume = consts.tile([1, 8], F32, name="dume", tag="dume")
                    nc.scalar.activation(
                        out=dume[:], in_=ones_bc_f[:, 0:8],
                        func=mybir.ActivationFunctionType.Exp, scale=1.0)

            def x_tr(j):
                xt_ps = ptr.tile([B_PER_CORE * C_IN, P], F32, tag="tr",
                                 name=f"xt_ps_{j}")
                nc.tensor.transpose(xt_ps[:], x_nat2[:, j, :, :], identity[:])
                for bb in range(B_PER_CORE):
                    src = xt_ps[bb * C_IN:(bb + 1) * C_IN, :]
                    dst = xTs[bb][0:C_IN, j * P:(j + 1) * P]
                    if bb == 0:
                        nc.vector.tensor_copy(out=dst, in_=src)
                    else:
                        nc.scalar.copy(out=dst, in_=src)

            def qk_unit(bb, name, s, relu_eng, pool, ptag):
                if s == 0:
                    t = perb.tile([D, N_TOK], F32R, name=f"{name}T_{bb}",
                                  tag=f"{name}T_{bb}", bufs=1)
                    (qTs if name == "q" else kTs)[bb] = t
                t = (qTs if name == "q" else kTs)[bb]
                ps = pool.tile([P, 1024], F32, tag=ptag,
                               name=f"pj_{bb}_{name}_{s}")
                for h in range(2):
                    nc.tensor.matmul(
                        ps[:, h * 512:(h + 1) * 512], w_sb[name][:],
                        xTs[bb][:, s * 1024 + h * 512:s * 1024 + (h + 1) * 512],
                        start=True, stop=True)
                dst = t[:, s * 1024:(s + 1) * 1024]
                if relu_eng == "act":
                    nc.scalar.activation(
                        out=dst, in_=ps[:],
                        func=mybir.ActivationFunctionType.Relu, scale=1.0)
                else:
                    nc.vector.tensor_scalar_max(dst, ps[:], 0.0)

            def v_dir(bb, j, relu_eng, pool, ptag):
                vp = pool.tile([P, P], F32, tag=ptag, name=f"vp_{bb}_{j}")
                nc.tensor.matmul(vp[:], xTs[bb][:, j * P:(j + 1) * P],
                                 w_sb["v"][:], start=True, stop=True)
                if relu_eng == "act":
                    nc.scalar.activation(
                        out=v8s[bb][:, j, :], in_=vp[:],
                        func=mybir.ActivationFunctionType.Relu, scale=1.0)
                else:
                    nc.vector.tensor_scalar_max(v8s[bb][:, j, :], vp[:], 0.0)

            for j in range(N_TILES):
                x_tr(j)
                warm_mm()
                if j >= 2:
                    for bb in range(B_PER_CORE):
                        v_dir(bb, j - 2, "vec", ptr, "tr")
                if j == 8:
                    qk_unit(0, "q", 0, "act", ppj, "pj")
                if j == 10:
                    qk_unit(0, "k", 0, "vec", ppj, "pj")
                if j == 12:
                    qk_unit(1, "q", 0, "act", ppj, "pj")
                if j == 14:
                    qk_unit(1, "k", 0, "vec", ppj, "pj")
            qk_unit(0, "q", 1, "act", ppj, "pj")
            warm_mm()
            qk_unit(0, "k", 1, "vec", ppj, "pj")
            warm_mm()
            qk_unit(1, "q", 1, "act", ppj, "pj")
            warm_mm()
            qk_unit(1, "k", 1, "vec", ppj, "pj")
            for j in range(N_TILES - 2, N_TILES):
                for bb in range(B_PER_CORE):
                    v_dir(bb, j, "vec", ptr, "tr")

        # ---------------- Phase B: attention sweeps ----------------
        pst = ctx.enter_context(tc.tile_pool(name="pst", bufs=2, space="PSUM"))
        pacc = ctx.enter_context(tc.tile_pool(name="pacc", bufs=1, space="PSUM"))
        pden = ctx.enter_context(tc.tile_pool(name="pden", bufs=1, space="PSUM"))

        def v_dir_pb(bb, j, eng):
            v_dir(bb, j, eng, pst, "st")

        def qk_unit_pb(bb, name, s, eng):
            qk_unit(bb, name, s, eng, pst, "st")

        # E pairs byte-interleaved: e8i[p, mhat, q, o], o = m%2
        e8i = epool.tile([P, MHAT, N_CHUNK, 2], FP8, tag="e8i", name="e8i")

        pending = []

        def emit_epilogue(prev, acc_prev):
            b_, c_, den_sb = prev
            rb = ep.tile([P, N_CHUNK], F32, tag="rb", name=f"rb_{b_}_{c_}")
            bc = pst.tile([P, N_CHUNK], F32, tag="st", name=f"bc_{b_}_{c_}")
            for h in range(2):
                cols = slice(h * 512, (h + 1) * 512)
                nc.tensor.matmul(bc[:, cols], ones_bc[:], den_sb[:, cols],
                                 start=True, stop=True)
            nc.vector.reciprocal_approx_fast(out=rb[:], in_=bc[:])
            o_sb = ep.tile([P, N_CHUNK], F32, tag="o_sb", name=f"o_{b_}_{c_}")
            w = N_CHUNK // 4
            for qq in range(4):
                cols = slice(qq * w, (qq + 1) * w)
                nc.vector.tensor_tensor(
                    o_sb[:, cols], acc_prev[:, cols], rb[:, cols],
                    mybir.AluOpType.mult)
                nc.sync.dma_start(
                    out=out[b_, :, c_ * N_CHUNK + qq * w:
                            c_ * N_CHUNK + (qq + 1) * w],
                    in_=o_sb[:, cols],
                )

        for b in range(B_PER_CORE):
            for chunk in range(N_CHUNKS):
                n0 = chunk * N_CHUNK
                qT, kT, v8 = qTs[b], kTs[b], v8s[b]
                acc = pacc.tile([P, N_CHUNK], F32, tag="acc",
                                name=f"acc_{b}_{chunk}")
                den = pden.tile([1, N_CHUNK], F32, tag="den",
                                name=f"den_{b}_{chunk}")

                # PV lags 2 pairs behind scores/exp, den lags 3
                for mh in range(MHAT + 3):
                    if mh < MHAT:
                        m0, m1 = 2 * mh, 2 * mh + 1
                        st0 = pst.tile([P, N_CHUNK], F32, tag="st",
                                       name=f"st_{b}_{chunk}_{m0}")
                        st1 = pst.tile([P, N_CHUNK], F32, tag="st",
                                       name=f"st_{b}_{chunk}_{m1}")
                        for m, st in ((m0, st0), (m1, st1)):
                            for h in range(2):
                                nc.tensor.matmul(
                                    st[:, h * 512:(h + 1) * 512],
                                    kT[:, m * P:(m + 1) * P],
                                    qT[:, n0 + h * 512:n0 + (h + 1) * 512],
                                    start=True, stop=True)
                            if EXP_ON_ACT[m]:
                                nc.scalar.activation(
                                    out=e8i[:, mh, :, m % 2], in_=st[:],
                                    func=mybir.ActivationFunctionType.Exp,
                                    scale=SCALE)
                            else:
                                nc.vector.tensor_scalar(
                                    out=e8i[:, mh, :, m % 2].bitcast(I8),
                                    in0=st[:],
                                    scalar1=LOG2E, scalar2=B8,
                                    op0=mybir.AluOpType.mult,
                                    op1=mybir.AluOpType.add)
                    if mh == 2 and pending:
                        emit_epilogue(*pending.pop())
                    mp = mh - 2
                    if 0 <= mp < MHAT:
                        for h in range(2):
                            nc.tensor.matmul(
                                acc[:, h * 512:(h + 1) * 512],
                                v8[:, 2 * mp:2 * mp + 2, :],
                                pair_ap(e8i[:, mp, h * 512:(h + 1) * 512, :]),
                                start=(mp == 0), stop=(mp == MHAT - 1),
                                perf_mode=DR)
                    md = mh - 3
                    if 0 <= md < MHAT:
                        for h in range(2):
                            nc.tensor.matmul(
                                den[:, h * 512:(h + 1) * 512],
                                ones8[:, :, 0:1],
                                pair_ap(e8i[:, md, h * 512:(h + 1) * 512, :]),
                                start=(md == 0), stop=(md == MHAT - 1),
                                perf_mode=DR)

                den_sb = ep.tile([1, N_CHUNK], F32R, tag="den_sb",
                                 name=f"den_sb_{b}_{chunk}")
                nc.vector.tensor_copy(out=den_sb[:], in_=den[:])
                pending.append(((b, chunk, den_sb), acc))

        emit_epilogue(*pending.pop())


_NC_CACHE = None


def _get_program():
    global _NC_CACHE
    if _NC_CACHE is None:
        _NC_CACHE = build_program()
    return _NC_CACHE


def kernel(x, Wq, bq, Wk, bk, Wv, bv, _trace=False):
    x = np.ascontiguousarray(np.asarray(x, dtype=np.float32))
    full_b = x.shape[0]
    assert full_b == N_CORES * B_PER_CORE, x.shape
    nc = _get_program()
    common = {
        "Wq": np.ascontiguousarray(np.asarray(Wq, np.float32)),
        "bq": np.ascontiguousarray(np.asarray(bq, np.float32)),
        "Wk": np.ascontiguousarray(np.asarray(Wk, np.float32)),
        "bk": np.ascontiguousarray(np.asarray(bk, np.float32)),
        "Wv": np.ascontiguousarray(np.asarray(Wv, np.float32)),
        "bv": np.ascontiguousarray(np.asarray(bv, np.float32)),
    }
    in_maps = [
        {"x": x[c * B_PER_CORE:(c + 1) * B_PER_CORE], **common}
        for c in range(N_CORES)
    ]
    res = run_bass_kernel_spmd(nc, in_maps, list(range(N_CORES)), trace=_trace)
    # device layout: out_T[b, d, ntilde], ntilde = j*128 + p -> token p*16+j
    outs = []
    for c in range(N_CORES):
        ot = res.results[c]["out"]  # [B_PER_CORE, D, N_TOK]
        ot = ot.reshape(B_PER_CORE, D, N_TILES, P)          # [b, d, j, p]
        ot = np.transpose(ot, (0, 3, 2, 1))                 # [b, p, j, d]
        outs.append(np.ascontiguousarray(
            ot.reshape(B_PER_CORE, N_TOK, D)))
    out_full = np.concatenate(outs, axis=0)
    if _trace:
        kernel.last_exec_time_ns = res.exec_time_ns
    return out_full


# revision 44
# speedup vs baseline: 1.1907x; 1.1907x over previous
"""Trainium2 Bass kernel for nn_Attention_53687091200195 (v3).

Reference computation (per batch b):
    Q = relu(x @ Wq + bq); K = relu(x @ Wk + bk); V = relu(x @ Wv + bv)
    S = Q @ K^T / sqrt(64); P = softmax(S, axis=-1); out = P @ V

Shapes: x [16, 2048, 64] f32, W* [64, 128] f32, b* [128] f32 -> out [16, 2048, 128].
Sharding: data-parallel over batch. 8 cores x 2 batches each; weights replicated.

v3 design (v0 baseline ~152us):
  - Token-permuted layout (internal token ntilde = j*128 + p maps to real
    token p*16 + j) keeps every DMA per-partition contiguous; attention is
    permutation-equivariant, the host undoes the permutation (pure layout).
  - Projections in f32r with bias folding (65-row contraction, ones row).
  - Scores S^T = K_m @ Q^T in f32r (16 m-tiles x 2 halves per 1024-q chunk).
  - E = exp(S/8) stored fp8e4m3 BYTE-INTERLEAVED in key-tile pairs:
    e8i[p, mhat, q, o] with o = m%2 at byte stride 2, so the DoubleRow
    matmuls' moving operand reads one 16-bit pair per lane per cycle ->
    2 fp8 MACs/cell/cycle.  PV (out^T += V_pair^T @ E_pair) and the softmax
    denominator (ones_pair^T @ E_pair) each take 16 512-col passes per chunk
    at ~2x the f32r MAC rate (vs 32 passes each in v0).
  - exp split across engines: ACT does exact exp->fp8 for 9/16 tiles per
    chunk, DVE synthesizes fp8 bits directly via a Schraudolph construction
    (int8(S*log2e + 55.63) IS the fp8e4m3 bit pattern of ~exp(S/8)) for 7.
  - No PE out-transposes: out^T [d, q] is normalized via a K=1 ones-matmul
    broadcast of den into PSUM, reciprocal_approx_fast, and tensor_tensor
    multiplies; stored transposed + permuted, host fixes layout.
  - den accumulates in a single PSUM bank ([33, 512] tile, query-half 1 at
    partition 32 via col-group tiling), freeing a bank for a dedicated
    broadcast pool so the epilogue never blocks the score pipeline.
  - PE warmup on zeroed f32r tiles + dummy exp during the initial x DMA so
    HAM reaches 2.4 GHz and the ACT exp table is resident before real work.
  - fp8 quantization of E/V adds ~0.6% output error (tolerance 2e-2).
"""

import numpy as np

import concourse.bass as bass
import concourse.mybir as mybir
import concourse.tile as tile
from concourse import bacc
from concourse.bass_utils import run_bass_kernel_spmd

N_CORES = 8
B_PER_CORE = 2
N_TOK = 2048
C_IN = 64
D = 128
P = 128
N_TILES = N_TOK // P          # 16
N_CHUNK = 1024
N_CHUNKS = N_TOK // N_CHUNK   # 2
MHAT = N_TILES // 2           # 8 pair-tiles per chunk
SCALE = 0.125                 # 1/sqrt(64)
LOG2E = 1.4426950408889634
B8 = 55.63                    # Schraudolph offset for fp8e4m3 bits

F32 = mybir.dt.float32
F32R = mybir.dt.float32r
FP8 = mybir.dt.float8e4
I8 = mybir.dt.int8
DR = mybir.MatmulPerfMode.DoubleRow

# exp engine per m-tile within a chunk: True -> ACT, False -> DVE (10/6
# split: strided DVE fp8 writes measure ~1.56us vs ACT's ~1.15us; the two
# both-ACT pairs sit apart so the pst rotation can absorb their serialization)
EXP_ON_ACT = [True, False] * (N_TILES // 2)
EXP_ON_ACT[1] = True
EXP_ON_ACT[15] = True


def pair_ap(e_slice):
    """Reorder a [128, N, 2] fp8 AP into DoubleRow's [K, 2, N] operand shape
    (dim1 = pair member at byte step 1, dim2 = columns at byte step 2)."""
    return bass.AP(
        tensor=e_slice.tensor,
        offset=e_slice.offset,
        ap=[e_slice.ap[0], e_slice.ap[2], e_slice.ap[1]],
    )


def build_program():
    nc = bacc.Bacc("TRN2", target_bir_lowering=False, debug=False,
                   num_devices=N_CORES)

    x = nc.dram_tensor("x", [B_PER_CORE, N_TOK, C_IN], F32, kind="ExternalInput").ap()
    wq = nc.dram_tensor("Wq", [C_IN, D], F32, kind="ExternalInput").ap()
    bq = nc.dram_tensor("bq", [D], F32, kind="ExternalInput").ap()
    wk = nc.dram_tensor("Wk", [C_IN, D], F32, kind="ExternalInput").ap()
    bk = nc.dram_tensor("bk", [D], F32, kind="ExternalInput").ap()
    wv = nc.dram_tensor("Wv", [C_IN, D], F32, kind="ExternalInput").ap()
    bv = nc.dram_tensor("bv", [D], F32, kind="ExternalInput").ap()
    # transposed + token-permuted output; host untangles the layout
    out = nc.dram_tensor("out", [B_PER_CORE, D, N_TOK], F32, kind="ExternalOutput").ap()

    with tile.TileContext(nc) as tc:
        kernel_body(tc, out, x, (wq, bq), (wk, bk), (wv, bv))

    nc.compile()
    return nc


def kernel_body(tc, out, x, qw, kw, vw):
    nc = tc.nc
    from contextlib import ExitStack
    ctx = ExitStack()
    with ctx:
        consts = ctx.enter_context(tc.tile_pool(name="consts", bufs=1))
        perb = ctx.enter_context(tc.tile_pool(name="perb", bufs=2))
        epool = ctx.enter_context(tc.tile_pool(name="epool", bufs=1))
        ep = ctx.enter_context(tc.tile_pool(name="ep", bufs=2))

        # --- warmup constants first: only DVE memsets in their deps ---
        warm_st = consts.tile([P, P], F32R, name="warm_st", tag="warm_st")
        nc.vector.memset(warm_st[:].bitcast(F32), 0.0)
        warm_mov = consts.tile([P, 512], F32R, name="warm_mov", tag="warm_mov")
        nc.vector.memset(warm_mov[:].bitcast(F32), 0.0)

        # --- constants ---
        identity = consts.tile([P, P], F32)
        nc.vector.memset(identity[:], 0.0)
        nc.gpsimd.affine_select(
            out=identity[:], in_=identity[:],
            compare_op=mybir.AluOpType.not_equal, fill=1.0,
            base=0, pattern=[[-1, P]], channel_multiplier=1)
        ones_bc_f = consts.tile([1, P], F32)
        nc.vector.memset(ones_bc_f[:], 1.0)
        ones_bc = consts.tile([1, P], F32R)
        nc.vector.tensor_copy(out=ones_bc[:], in_=ones_bc_f[:])
        ones8 = consts.tile([P, 2, 16], FP8)
        nc.vector.memset(ones8[:], 1.0)

        # x for both batches, token-permuted: x_nat2[p, j, b, c] = x[b, p*16+j, c]
        x_nat2 = consts.tile([P, N_TILES, B_PER_CORE, C_IN], F32, name="x_nat2",
                             tag="x_nat2")
        # 6 DMAs across 3 queues: x fully resident by ~6us so the transpose
        # stream never stalls mid-phase-A (which would re-throttle HAM)
        H = N_TILES // 3 if N_TILES % 3 == 0 else 6
        qengs = (nc.sync, nc.scalar, nc.gpsimd)
        bounds = [0, 6, 11, 16]
        for jh in range(3):
            j0, j1 = bounds[jh], bounds[jh + 1]
            for bb in range(B_PER_CORE):
                qengs[jh].dma_start(
                    out=x_nat2[:, j0:j1, bb, :],
                    in_=bass.AP(
                        tensor=x.tensor,
                        offset=bb * N_TOK * C_IN + j0 * C_IN,
                        ap=[[N_TILES * C_IN, P], [C_IN, j1 - j0], [1, C_IN]],
                    ),
                )

        # Bias folding: row 64 of xT is ones, row 64 of each weight is bias.
        w_sb = {}
        for name, (w, b) in (("q", qw), ("k", kw), ("v", vw)):
            wf = consts.tile([C_IN, D], F32, name=f"wf_{name}", tag=f"wf_{name}")
            nc.scalar.dma_start(out=wf[:], in_=w[:])
            bf = consts.tile([1, D], F32, name=f"bf_{name}", tag=f"bf_{name}")
            nc.scalar.dma_start(out=bf[:], in_=b[:])
            w2 = consts.tile([C_IN + 1, D], F32R, name=f"w_{name}", tag=f"w_{name}")
            nc.vector.tensor_copy(out=w2[0:C_IN, :], in_=wf[:])
            nc.vector.tensor_copy(out=w2[C_IN:C_IN + 1, :], in_=bf[:])
            w_sb[name] = w2

        xTs = [perb.tile([C_IN + 1, N_TOK], F32R, name=f"xT_{bb}",
                         tag=f"xT_{bb}", bufs=1)
               for bb in range(B_PER_CORE)]
        for bb in range(B_PER_CORE):
            nc.gpsimd.memset(xTs[bb][C_IN:C_IN + 1, :].bitcast(F32), 1.0)

        qTs, kTs = {}, {}
        v8s = [perb.tile([P, N_TILES, D], FP8, name=f"v8_{bb}",
                         tag=f"v8_{bb}", bufs=1)
               for bb in range(B_PER_CORE)]

        # ---------------- Phase A: prologue (own PSUM scope) ----------------
        with tc.tile_pool(name="ptr", bufs=4, space="PSUM") as ptr, \
             tc.tile_pool(name="ppj", bufs=2, space="PSUM") as ppj:

            # PE warmup: ~10 cold matmuls during the x DMA flip HAM to
            # K=8/8 (2.4 GHz) before the real stream begins.
            for i in range(10):
                warm = ptr.tile([P, 512], F32, tag="tr", name=f"warm{i}")
                nc.tensor.matmul(warm[:], warm_st[:], warm_mov[:],
                                 start=True, stop=True)
                if i == 0:
                    # dummy exp: ACT table load off the critical path
                    dume = consts.tile([1, 8], F32, name="dume", tag="dume")
                    nc.scalar.activation(
                        out=dume[:], in_=ones_bc_f[:, 0:8],
                        func=mybir.ActivationFunctionType.Exp, scale=1.0)

            def x_tr(j):
                xt_ps = ptr.tile([B_PER_CORE * C_IN, P], F32, tag="tr",
                                 name=f"xt_ps_{j}")
                nc.tensor.transpose(xt_ps[:], x_nat2[:, j, :, :], identity[:])
                for bb in range(B_PER_CORE):
                    src = xt_ps[bb * C_IN:(bb + 1) * C_IN, :]
                    dst = xTs[bb][0:C_IN, j * P:(j + 1) * P]
                    if bb == 0:
                        nc.vector.tensor_copy(out=dst, in_=src)
                    else:
                        nc.scalar.copy(out=dst, in_=src)

            def qk_unit(bb, name, s, relu_eng, pool, ptag):
                if s == 0:
                    t = perb.tile([D, N_TOK], F32R, name=f"{name}T_{bb}",
                                  tag=f"{name}T_{bb}", bufs=1)
                    (qTs if name == "q" else kTs)[bb] = t
                t = (qTs if name == "q" else kTs)[bb]
                ps = pool.tile([P, 1024], F32, tag=ptag,
                               name=f"pj_{bb}_{name}_{s}")
                for h in range(2):
                    nc.tensor.matmul(
                        ps[:, h * 512:(h + 1) * 512], w_sb[name][:],
                        xTs[bb][:, s * 1024 + h * 512:s * 1024 + (h + 1) * 512],
                        start=True, stop=True)
                dst = t[:, s * 1024:(s + 1) * 1024]
                if relu_eng == "act":
                    nc.scalar.activation(
                        out=dst, in_=ps[:],
                        func=mybir.ActivationFunctionType.Relu, scale=1.0)
                else:
                    nc.vector.tensor_scalar_max(dst, ps[:], 0.0)

            def v_dir(bb, j, relu_eng, pool, ptag):
                vp = pool.tile([P, P], F32, tag=ptag, name=f"vp_{bb}_{j}")
                nc.tensor.matmul(vp[:], xTs[bb][:, j * P:(j + 1) * P],
                                 w_sb["v"][:], start=True, stop=True)
                if relu_eng == "act":
                    nc.scalar.activation(
                        out=v8s[bb][:, j, :], in_=vp[:],
                        func=mybir.ActivationFunctionType.Relu, scale=1.0)
                else:
                    nc.vector.tensor_scalar_max(v8s[bb][:, j, :], vp[:], 0.0)

            for j in range(N_TILES):
                x_tr(j)
                if j >= 2:
                    for bb in range(B_PER_CORE):
                        v_dir(bb, j - 2, "vec", ptr, "tr")
                if j == 8:
                    qk_unit(0, "q", 0, "act", ppj, "pj")
                if j == 10:
                    qk_unit(0, "k", 0, "vec", ppj, "pj")
                if j == 12:
                    qk_unit(1, "q", 0, "act", ppj, "pj")
                if j == 14:
                    qk_unit(1, "k", 0, "vec", ppj, "pj")
            qk_unit(0, "q", 1, "act", ppj, "pj")
            qk_unit(0, "k", 1, "vec", ppj, "pj")
            qk_unit(1, "q", 1, "act", ppj, "pj")
            qk_unit(1, "k", 1, "vec", ppj, "pj")
            for j in range(N_TILES - 2, N_TILES):
                for bb in range(B_PER_CORE):
                    v_dir(bb, j, "vec", ptr, "tr")

        # ---------------- Phase B: attention sweeps ----------------
        pst = ctx.enter_context(tc.tile_pool(name="pst", bufs=2, space="PSUM"))
        pacc = ctx.enter_context(tc.tile_pool(name="pacc", bufs=1, space="PSUM"))
        pden = ctx.enter_context(tc.tile_pool(name="pden", bufs=1, space="PSUM"))

        def v_dir_pb(bb, j, eng):
            v_dir(bb, j, eng, pst, "st")

        def qk_unit_pb(bb, name, s, eng):
            qk_unit(bb, name, s, eng, pst, "st")

        # E pairs byte-interleaved: e8i[p, mhat, q, o], o = m%2
        e8i = epool.tile([P, MHAT, N_CHUNK, 2], FP8, tag="e8i", name="e8i")

        pending = []

        def emit_epilogue(prev, acc_prev):
            b_, c_, den_sb = prev
            rb = ep.tile([P, N_CHUNK], F32, tag="rb", name=f"rb_{b_}_{c_}")
            bc = pst.tile([P, N_CHUNK], F32, tag="st", name=f"bc_{b_}_{c_}")
            for h in range(2):
                cols = slice(h * 512, (h + 1) * 512)
                nc.tensor.matmul(bc[:, cols], ones_bc[:], den_sb[:, cols],
                                 start=True, stop=True)
            nc.vector.reciprocal_approx_fast(out=rb[:], in_=bc[:])
            o_sb = ep.tile([P, N_CHUNK], F32, tag="o_sb", name=f"o_{b_}_{c_}")
            w = N_CHUNK // 4
            for qq in range(4):
                cols = slice(qq * w, (qq + 1) * w)
                nc.vector.tensor_tensor(
                    o_sb[:, cols], acc_prev[:, cols], rb[:, cols],
                    mybir.AluOpType.mult)
                nc.sync.dma_start(
                    out=out[b_, :, c_ * N_CHUNK + qq * w:
                            c_ * N_CHUNK + (qq + 1) * w],
                    in_=o_sb[:, cols],
                )

        for b in range(B_PER_CORE):
            for chunk in range(N_CHUNKS):
                n0 = chunk * N_CHUNK
                qT, kT, v8 = qTs[b], kTs[b], v8s[b]
                acc = pacc.tile([P, N_CHUNK], F32, tag="acc",
                                name=f"acc_{b}_{chunk}")
                den = pden.tile([1, N_CHUNK], F32, tag="den",
                                name=f"den_{b}_{chunk}")

                # PV lags 2 pairs behind scores/exp, den lags 3
                for mh in range(MHAT + 3):
                    if mh < MHAT:
                        m0, m1 = 2 * mh, 2 * mh + 1
                        st0 = pst.tile([P, N_CHUNK], F32, tag="st",
                                       name=f"st_{b}_{chunk}_{m0}")
                        st1 = pst.tile([P, N_CHUNK], F32, tag="st",
                                       name=f"st_{b}_{chunk}_{m1}")
                        for m, st in ((m0, st0), (m1, st1)):
                            for h in range(2):
                                nc.tensor.matmul(
                                    st[:, h * 512:(h + 1) * 512],
                                    kT[:, m * P:(m + 1) * P],
                                    qT[:, n0 + h * 512:n0 + (h + 1) * 512],
                                    start=True, stop=True)
                            if EXP_ON_ACT[m]:
                                nc.scalar.activation(
                                    out=e8i[:, mh, :, m % 2], in_=st[:],
                                    func=mybir.ActivationFunctionType.Exp,
                                    scale=SCALE)
                            else:
                                nc.vector.tensor_scalar(
                                    out=e8i[:, mh, :, m % 2].bitcast(I8),
                                    in0=st[:],
                                    scalar1=LOG2E, scalar2=B8,
                                    op0=mybir.AluOpType.mult,
                                    op1=mybir.AluOpType.add)
                    if mh == 2 and pending:
                        emit_epilogue(*pending.pop())
                    mp = mh - 2
                    if 0 <= mp < MHAT:
                        for h in range(2):
                            nc.tensor.matmul(
                                acc[:, h * 512:(h + 1) * 512],
                                v8[:, 2 * mp:2 * mp + 2, :],
                                pair_ap(e8i[:, mp, h * 512:(h + 1) * 512, :]),
                                start=(mp == 0), stop=(mp == MHAT - 1),
                                perf_mode=DR)
                    md = mh - 3
                    if 0 <= md < MHAT:
                        for h in range(2):
                            nc.tensor.matmul(
                                den[:, h * 512:(h + 1) * 512],
                                ones8[:, :, 0:1],
                                pair_ap(e8i[:, md, h * 512:(h + 1) * 512, :]),
                                start=(md == 0), stop=(md == MHAT - 1),
                                perf_mode=DR)

                den_sb = ep.tile([1, N_CHUNK], F32R, tag="den_sb",
                                 name=f"den_sb_{b}_{chunk}")
                # Relu is an identity here (den > 0) and, unlike Copy, is a
                # walrus-accepted f32r producer; frees ~1.2us/chunk of DVE
                nc.scalar.activation(
                    out=den_sb[:], in_=den[:],
                    func=mybir.ActivationFunctionType.Relu, scale=1.0)
                pending.append(((b, chunk, den_sb), acc))

        emit_epilogue(*pending.pop())


_NC_CACHE = None


def _get_program():
    global _NC_CACHE
    if _NC_CACHE is None:
        _NC_CACHE = build_program()
    return _NC_CACHE


def kernel(x, Wq, bq, Wk, bk, Wv, bv, _trace=False):
    x = np.ascontiguousarray(np.asarray(x, dtype=np.float32))
    full_b = x.shape[0]
    assert full_b == N_CORES * B_PER_CORE, x.shape
    nc = _get_program()
    common = {
        "Wq": np.ascontiguousarray(np.asarray(Wq, np.float32)),
        "bq": np.ascontiguousarray(np.asarray(bq, np.float32)),
        "Wk": np.ascontiguousarray(np.asarray(Wk, np.float32)),
        "bk": np.ascontiguousarray(np.asarray(bk, np.float32)),
        "Wv": np.ascontiguousarray(np.asarray(Wv, np.float32)),
        "bv": np.ascontiguousarray(np.asarray(bv, np.float32)),
    }
    in_maps = [
        {"x": x[c * B_PER_CORE:(c + 1) * B_PER_CORE], **common}
        for c in range(N_CORES)
    ]
    res = run_bass_kernel_spmd(nc, in_maps, list(range(N_CORES)), trace=_trace)
    # device layout: out_T[b, d, ntilde], ntilde = j*128 + p -> token p*16+j
    outs = []
    for c in range(N_CORES):
        ot = res.results[c]["out"]  # [B_PER_CORE, D, N_TOK]
        ot = ot.reshape(B_PER_CORE, D, N_TILES, P)          # [b, d, j, p]
        ot = np.transpose(ot, (0, 3, 2, 1))                 # [b, p, j, d]
        outs.append(np.ascontiguousarray(
            ot.reshape(B_PER_CORE, N_TOK, D)))
    out_full = np.concatenate(outs, axis=0)
    if _trace:
        kernel.last_exec_time_ns = res.exec_time_ns
    return out_full
